# revision 18
# baseline (speedup 1.0000x reference)
"""AdderNet depthwise 3x3 L1-distance conv for Trainium2, 8-core data parallel.

out[b,c,h,w] = -sum_{i,j in 3x3} |x_pad[b,c,h+i,w+j] - W[c,0,i,j]|

Strategy (per core, 4 images of the batch = 16 (b,c) planes):
- Host zero-pads each [512,512] plane to [514,514]; pads ARE semantic
  (reference pads with zeros inside the |.| sum).
- Per plane: 4 row-blocks of 128 rows live on the 128 SBUF partitions
  (block index on the free dim). W-shifts (j) are free-dim offsets.
- 9 abs-diff tap planes |x - w[c,i,j]| in bf16: 6 on VectorE via fused
  tensor_scalar(subtract, abs_max), 3 on ScalarE via activation(Abs, bias).
- H-shifts (i) + the 9-tap sum run on TensorE: 3 shifted-identity bf16
  matrices as stationary operands, 9 accumulating matmuls per PSUM bank.
- ScalarE evacuates PSUM -> SBUF with scale=-1 (the output negation).
- Row-block seams (2 rows per 128-block) are patched by 2 small fixup
  tiles that pack 4-row bands around every seam for 8 images each.
"""

import numpy as np
import ml_dtypes

B, C, H, W = 32, 4, 512, 512
N_CORES = 8
B_LOC = B // N_CORES          # 4 images per core
N_IMG = B_LOC * C             # 16 (b,c) planes per core
HP, WP = H + 2, W + 2         # 514, 514
NBLK = 4                      # row blocks of 128 per plane (rows 0..511 of padded)
P = 128

# tap t = 3*i + j ; taps computed on ScalarE (rest on VectorE custom absdiff)
ACT_TAPS = (0, 2, 4, 6, 8)
# psum blocks evacuated by VectorE (rest by ScalarE)
DVE_EVAC_BLOCKS = 2

_PROGRAM_CACHE = {}


def _register_absdiff():
    """Register a custom DVE op: out = |in0 - s0| = relu(in0-s0) + relu(s0-in0).
    tensor_scalar has no legal fused abs on TRN2 (abs_max fails the ISA check,
    arith+bitwise ops can't mix), so this 1-instruction DVE op is the cheapest
    legal per-tap absdiff."""
    from concourse import dve_ops
    from concourse.dve_spec import Spec, Src0, C0, relu, lower
    from concourse.dve_uop import DveOpSpec

    for o in dve_ops.OPS:
        if o.name == "ABS_DIFF_ANT":
            return o
    def _ref(in0, in1, s0, s1, imm2):
        s = np.asarray(s0)
        if s.ndim and in0.ndim > s.ndim:  # [P,1] scalar vs [P,S,N] tensor
            s = s.reshape(s.shape[0], *([1] * (in0.ndim - 1)))
        return np.abs(in0.astype(np.float32) - s).astype(np.float32)

    spec = Spec(
        body=relu(Src0 - C0) + relu(C0 - Src0),
        reference=_ref,
    )
    shas = {
        ver: DveOpSpec(name="ABS_DIFF_ANT", uops=lower(spec, ver=ver)).sha(ver)
        for ver in ("v3", "v4")
    }
    op = dve_ops.DveOp("ABS_DIFF_ANT", spec, subdim=False, uops_sha=shas)
    dve_ops.OPS.append(op)
    dve_ops.CUSTOM_DVE_SPECS[op.name] = spec
    dve_ops._SUB_OPCODE_FOR_NAME[op.name] = dve_ops._CUSTOM_DVE_ROW_BASE + len(dve_ops.OPS) - 1
    return op


def _build_program():
    import concourse.mybir as mybir
    import concourse.tile as tile
    from concourse import bacc

    f32 = mybir.dt.float32
    bf16 = mybir.dt.bfloat16
    absdiff = _register_absdiff()
    nc = bacc.Bacc("TRN2", target_bir_lowering=False)

    xpad = nc.declare_dram_parameter("xpad", [N_IMG, HP, WP], f32, isOutput=False)
    smat = nc.declare_dram_parameter("smat", [3, P, P], bf16, isOutput=False)
    # bias[:, st*18 + t] : +w for DVE taps (cols 0-8), -w for ACT taps (9-17);
    # supertiles 0-15 then fixup tiles 16-17
    bias = nc.declare_dram_parameter("bias", [P, (N_IMG + 2) * 18], f32, isOutput=False)
    outp = nc.declare_dram_parameter("outp", [N_IMG, HP, W], f32, isOutput=True)

    with tile.TileContext(nc) as tc:
        with (
            tc.tile_pool(name="const", bufs=1) as cpool,
            tc.tile_pool(name="xp", bufs=3) as xpool,
            tc.tile_pool(name="dp", bufs=14) as dpool,
            tc.tile_pool(name="op", bufs=3) as opool,
            tc.tile_pool(name="ps", bufs=2, space="PSUM") as ppool,
        ):
            s_t = cpool.tile([P, 3, P], bf16, tag="s")
            nc.sync.dma_start(out=s_t, in_=smat[:].rearrange("s k p -> k s p"))
            b_all = cpool.tile([P, (N_IMG + 2) * 18], f32, tag="ball")
            nc.sync.dma_start(out=b_all, in_=bias[:])

            # Warmup activations with minimal deps so walrus attaches the ACT
            # table-load pseudo-instruction here (a loaded instruction with 2
            # DMA waits + table load exceeds the sync-wait slots).
            warm = cpool.tile([P, 2], f32, tag="warm")
            nc.vector.memset(warm, 0.0)
            nc.scalar.activation(
                out=warm[:, 0:1],
                in_=warm[:, 1:2],
                func=mybir.ActivationFunctionType.Abs,
                bias=0.0,
                scale=1.0,
            )
            nc.scalar.activation(
                out=warm[:, 1:2],
                in_=warm[:, 0:1],
                func=mybir.ActivationFunctionType.Copy,
                scale=-1.0,
            )

            # ---- main supertiles: one per (b,c) plane ----
            for st in range(N_IMG):
                x_t = xpool.tile([P, NBLK, WP], f32, tag="x")
                nc.sync.dma_start(
                    out=x_t,
                    in_=xpad[st, 0:512, :].rearrange("(b q) w -> q b w", q=P),
                )
                bofs = st * 18
                d_tiles = []
                for t in range(9):
                    i, j = divmod(t, 3)
                    d = dpool.tile([P, NBLK, W], bf16, tag="d")
                    src = x_t[:, :, j : j + W]
                    if t in ACT_TAPS:
                        nc.scalar.activation(
                            out=d,
                            in_=src,
                            func=mybir.ActivationFunctionType.Abs,
                            bias=b_all[:, bofs + 9 + t : bofs + 10 + t],
                            scale=1.0,
                        )
                    else:
                        nc.vector._custom_dve(
                            absdiff,
                            out=d,
                            in0=src,
                            s0=b_all[:, bofs + t : bofs + t + 1],
                        )
                    d_tiles.append(d)

                ps = ppool.tile([P, NBLK, W], f32, tag="ps")
                for i in range(3):
                    for j in range(3):
                        t = 3 * i + j
                        for blk in range(NBLK):
                            nc.tensor.matmul(
                                ps[:, blk, :],
                                lhsT=s_t[:, i, :],
                                rhs=d_tiles[t][:, blk, :],
                                start=(t == 0),
                                stop=(t == 8),
                            )

                o_t = opool.tile([P, NBLK, W], f32, tag="o")
                eb = DVE_EVAC_BLOCKS
                nc.vector.tensor_scalar(
                    out=o_t[:, :eb],
                    in0=ps[:, :eb],
                    scalar1=-1.0,
                    scalar2=None,
                    op0=mybir.AluOpType.mult,
                )
                nc.scalar.activation(
                    out=o_t[:, eb:],
                    in_=ps[:, eb:],
                    func=mybir.ActivationFunctionType.Copy,
                    scale=-1.0,
                )
                nc.sync.dma_start(
                    out=outp[st, 0:512, :].rearrange("(b q) w -> q b w", q=P)[1:127],
                    in_=o_t[1:127],
                )

            # ---- fixup tiles: rows 127,128,255,256,383,384,511,512 (padded
            # coords) of each plane, 8 planes per tile. Partition layout:
            # q = 16*g + 4*band + r ; band b covers padded rows 126+128b .. 129+128b
            for fi in range(2):
                g0 = fi * 8
                xf = xpool.tile([P, WP], f32, tag="xf")
                for band in range(4):
                    nc.sync.dma_start(
                        out=xf[32 * band : 32 * (band + 1)],
                        in_=xpad[g0 : g0 + 8, 126 + 128 * band : 130 + 128 * band, :],
                    )
                bofs = (N_IMG + fi) * 18
                df_tiles = []
                for t in range(9):
                    i, j = divmod(t, 3)
                    d = dpool.tile([P, W], bf16, tag="d")
                    src = xf[:, j : j + W]
                    if t in ACT_TAPS:
                        nc.scalar.activation(
                            out=d,
                            in_=src,
                            func=mybir.ActivationFunctionType.Abs,
                            bias=b_all[:, bofs + 9 + t : bofs + 10 + t],
                            scale=1.0,
                        )
                    else:
                        nc.vector._custom_dve(
                            absdiff,
                            out=d,
                            in0=src,
                            s0=b_all[:, bofs + t : bofs + t + 1],
                        )
                    df_tiles.append(d)

                pf = ppool.tile([P, W], mybir.dt.float32, tag="ps")
                for i in range(3):
                    t0 = 3 * i
                    for j in range(3):
                        nc.tensor.matmul(
                            pf,
                            lhsT=s_t[:, i, :],
                            rhs=df_tiles[t0 + j],
                            start=(t0 + j == 0),
                            stop=(t0 + j == 8),
                        )

                of = opool.tile([P, W], f32, tag="o")
                nc.scalar.activation(
                    out=of,
                    in_=pf,
                    func=mybir.ActivationFunctionType.Copy,
                    scale=-1.0,
                )
                for band in range(4):
                    for g in range(8):
                        lo = 32 * band + 4 * g
                        nc.sync.dma_start(
                            out=outp[g0 + g, 127 + 128 * band : 129 + 128 * band, :],
                            in_=of[lo + 1 : lo + 3],
                        )
    nc.finalize()
    return nc


def _get_program():
    if "nc" not in _PROGRAM_CACHE:
        _PROGRAM_CACHE["nc"] = _build_program()
    return _PROGRAM_CACHE["nc"]


def _host_consts(weight):
    """Shift matrices + per-partition bias tables (shared by all cores)."""
    w9 = np.asarray(weight, np.float32).reshape(C, 9)  # [c, t]

    S = np.zeros((3, P, P), np.float32)
    for i in range(3):
        for p in range(P):
            k = p + i - 1
            if 0 <= k < P:
                S[i, k, p] = 1.0
    S = S.astype(ml_dtypes.bfloat16)

    # one preloaded bias table [P, (N_IMG+2)*18]:
    # cols st*18+t = +w (DVE taps), st*18+9+t = -w (ACT taps)
    bias = np.zeros((P, (N_IMG + 2) * 18), np.float32)
    for st in range(N_IMG):  # main tiles: channel st % C, all partitions equal
        c = st % C
        bias[:, st * 18 : st * 18 + 9] = w9[c][None, :]
        bias[:, st * 18 + 9 : st * 18 + 18] = -w9[c][None, :]
    for fi in range(2):  # fixup tiles: partition q = 32*band + 4*g + r
        o = (N_IMG + fi) * 18
        for band in range(4):
            for g in range(8):
                c = (fi * 8 + g) % C
                lo = 32 * band + 4 * g
                bias[lo : lo + 4, o : o + 9] = w9[c][None, :]
                bias[lo : lo + 4, o + 9 : o + 18] = -w9[c][None, :]
    return S, bias


def kernel(input, weight):
    from concourse.bass_utils import run_bass_kernel_spmd

    x = np.asarray(input, np.float32)
    S, bias = _host_consts(weight)

    xpad = np.pad(x, ((0, 0), (0, 0), (1, 1), (1, 1)))  # [B, C, HP, WP]
    in_maps = []
    for core in range(N_CORES):
        shard = np.ascontiguousarray(
            xpad[core * B_LOC : (core + 1) * B_LOC].reshape(N_IMG, HP, WP)
        )
        in_maps.append({"xpad": shard, "smat": S, "bias": bias})

    nc = _get_program()
    res = run_bass_kernel_spmd(nc, in_maps, core_ids=list(range(N_CORES)))

    out = np.empty((B, C, H, W), np.float32)
    for core in range(N_CORES):
        o = res.results[core]["outp"].reshape(B_LOC, C, HP, W)
        out[core * B_LOC : (core + 1) * B_LOC] = o[:, :, 1 : H + 1, :]
    return out


# revision 20
# speedup vs baseline: 1.0031x; 1.0031x over previous
"""AdderNet depthwise 3x3 L1-distance conv for Trainium2, 8-core data parallel.

out[b,c,h,w] = -sum_{i,j in 3x3} |x_pad[b,c,h+i,w+j] - W[c,0,i,j]|

Strategy (per core, 4 images of the batch = 16 (b,c) planes):
- Host zero-pads each [512,512] plane to [514,514]; pads ARE semantic
  (reference pads with zeros inside the |.| sum).
- Per plane: 4 row-blocks of 128 rows live on the 128 SBUF partitions
  (block index on the free dim). W-shifts (j) are free-dim offsets.
- 9 abs-diff tap planes |x - w[c,i,j]| in bf16: 6 on VectorE via fused
  tensor_scalar(subtract, abs_max), 3 on ScalarE via activation(Abs, bias).
- H-shifts (i) + the 9-tap sum run on TensorE: 3 shifted-identity bf16
  matrices as stationary operands, 9 accumulating matmuls per PSUM bank.
- ScalarE evacuates PSUM -> SBUF with scale=-1 (the output negation).
- Row-block seams (2 rows per 128-block) are patched by 2 small fixup
  tiles that pack 4-row bands around every seam for 8 images each.
"""

import numpy as np
import ml_dtypes

B, C, H, W = 32, 4, 512, 512
N_CORES = 8
B_LOC = B // N_CORES          # 4 images per core
N_IMG = B_LOC * C             # 16 (b,c) planes per core
HP, WP = H + 2, W + 2         # 514, 514
NBLK = 4                      # row blocks of 128 per plane (rows 0..511 of padded)
P = 128

# tap t = 3*i + j ; taps computed on ScalarE (rest on VectorE custom absdiff)
ACT_TAPS = (0, 2, 4, 6, 8)
# psum blocks evacuated by VectorE (rest by ScalarE)
DVE_EVAC_BLOCKS = 2

_PROGRAM_CACHE = {}


def _register_absdiff():
    """Register a custom DVE op: out = |in0 - s0| = relu(in0-s0) + relu(s0-in0).
    tensor_scalar has no legal fused abs on TRN2 (abs_max fails the ISA check,
    arith+bitwise ops can't mix), so this 1-instruction DVE op is the cheapest
    legal per-tap absdiff."""
    from concourse import dve_ops
    from concourse.dve_spec import Spec, Src0, C0, relu, lower
    from concourse.dve_uop import DveOpSpec

    for o in dve_ops.OPS:
        if o.name == "ABS_DIFF_ANT":
            return o
    def _ref(in0, in1, s0, s1, imm2):
        s = np.asarray(s0)
        if s.ndim and in0.ndim > s.ndim:  # [P,1] scalar vs [P,S,N] tensor
            s = s.reshape(s.shape[0], *([1] * (in0.ndim - 1)))
        return np.abs(in0.astype(np.float32) - s).astype(np.float32)

    spec = Spec(
        body=relu(Src0 - C0) + relu(C0 - Src0),
        reference=_ref,
    )
    shas = {
        ver: DveOpSpec(name="ABS_DIFF_ANT", uops=lower(spec, ver=ver)).sha(ver)
        for ver in ("v3", "v4")
    }
    op = dve_ops.DveOp("ABS_DIFF_ANT", spec, subdim=False, uops_sha=shas)
    dve_ops.OPS.append(op)
    dve_ops.CUSTOM_DVE_SPECS[op.name] = spec
    dve_ops._SUB_OPCODE_FOR_NAME[op.name] = dve_ops._CUSTOM_DVE_ROW_BASE + len(dve_ops.OPS) - 1
    return op


def _patch_ldw_opt():
    """walrus dedups back-to-back LDWEIGHTS of the same stationary tensor
    only with --enable-ldw-opt; concourse hardcodes it off. Our inner loop
    issues 12 consecutive matmuls per stationary shift matrix, and the
    per-matmul reload serializes PE fill/drain (379ns/MM vs 216 target)."""
    import concourse.bass_utils as bu

    if getattr(bu, "_ldw_patched", False):
        return
    orig = bu.run_command

    def patched(argv, **kw):
        argv = [
            a
            for a in argv
        ]
        return orig(argv, **kw)

    bu.run_command = patched
    bu._ldw_patched = True


def _build_program():
    import concourse.mybir as mybir
    import concourse.tile as tile
    from concourse import bacc

    _patch_ldw_opt()

    f32 = mybir.dt.float32
    bf16 = mybir.dt.bfloat16
    absdiff = _register_absdiff()
    nc = bacc.Bacc("TRN2", target_bir_lowering=False)

    xpad = nc.declare_dram_parameter("xpad", [N_IMG, HP, WP], f32, isOutput=False)
    smat = nc.declare_dram_parameter("smat", [3, P, P], bf16, isOutput=False)
    # bias[:, st*18 + t] : +w for DVE taps (cols 0-8), -w for ACT taps (9-17);
    # supertiles 0-15 then fixup tiles 16-17
    bias = nc.declare_dram_parameter("bias", [P, (N_IMG + 2) * 18], f32, isOutput=False)
    outp = nc.declare_dram_parameter("outp", [N_IMG, HP, W], f32, isOutput=True)

    with tile.TileContext(nc) as tc:
        with (
            tc.tile_pool(name="const", bufs=1) as cpool,
            tc.tile_pool(name="xp", bufs=3) as xpool,
            tc.tile_pool(name="dp", bufs=16) as dpool,
            tc.tile_pool(name="op", bufs=4) as opool,
            tc.tile_pool(name="ps", bufs=2, space="PSUM") as ppool,
        ):
            s_t = cpool.tile([P, 3, P], bf16, tag="s")
            nc.sync.dma_start(out=s_t, in_=smat[:].rearrange("s k p -> k s p"))
            b_all = cpool.tile([P, (N_IMG + 2) * 18], f32, tag="ball")
            nc.sync.dma_start(out=b_all, in_=bias[:])

            # Warmup activations with minimal deps so walrus attaches the ACT
            # table-load pseudo-instruction here (a loaded instruction with 2
            # DMA waits + table load exceeds the sync-wait slots).
            warm = cpool.tile([P, 2], f32, tag="warm")
            nc.vector.memset(warm, 0.0)
            nc.scalar.activation(
                out=warm[:, 0:1],
                in_=warm[:, 1:2],
                func=mybir.ActivationFunctionType.Abs,
                bias=0.0,
                scale=1.0,
            )
            nc.scalar.activation(
                out=warm[:, 1:2],
                in_=warm[:, 0:1],
                func=mybir.ActivationFunctionType.Copy,
                scale=-1.0,
            )

            # ---- main supertiles: one per (b,c) plane ----
            for st in range(N_IMG):
                x_t = xpool.tile([P, NBLK, WP], f32, tag="x")
                nc.sync.dma_start(
                    out=x_t,
                    in_=xpad[st, 0:512, :].rearrange("(b q) w -> q b w", q=P),
                )
                bofs = st * 18
                d_tiles = []
                for t in range(9):
                    i, j = divmod(t, 3)
                    d = dpool.tile([P, NBLK, W], bf16, tag="d")
                    src = x_t[:, :, j : j + W]
                    if t in ACT_TAPS:
                        nc.scalar.activation(
                            out=d,
                            in_=src,
                            func=mybir.ActivationFunctionType.Abs,
                            bias=b_all[:, bofs + 9 + t : bofs + 10 + t],
                            scale=1.0,
                        )
                    else:
                        nc.vector._custom_dve(
                            absdiff,
                            out=d,
                            in0=src,
                            s0=b_all[:, bofs + t : bofs + t + 1],
                        )
                    d_tiles.append(d)

                ps = ppool.tile([P, NBLK, W], f32, tag="ps")
                for i in range(3):
                    for j in range(3):
                        t = 3 * i + j
                        for blk in range(NBLK):
                            nc.tensor.matmul(
                                ps[:, blk, :],
                                lhsT=s_t[:, i, :],
                                rhs=d_tiles[t][:, blk, :],
                                start=(t == 0),
                                stop=(t == 8),
                            )

                o_t = opool.tile([P, NBLK, W], f32, tag="o")
                eb = DVE_EVAC_BLOCKS
                nc.vector.tensor_scalar(
                    out=o_t[:, :eb],
                    in0=ps[:, :eb],
                    scalar1=-1.0,
                    scalar2=None,
                    op0=mybir.AluOpType.mult,
                )
                nc.scalar.activation(
                    out=o_t[:, eb:],
                    in_=ps[:, eb:],
                    func=mybir.ActivationFunctionType.Copy,
                    scale=-1.0,
                )
                nc.sync.dma_start(
                    out=outp[st, 0:512, :].rearrange("(b q) w -> q b w", q=P)[1:127],
                    in_=o_t[1:127],
                )

            # ---- fixup tiles: rows 127,128,255,256,383,384,511,512 (padded
            # coords) of each plane, 8 planes per tile. Partition layout:
            # q = 16*g + 4*band + r ; band b covers padded rows 126+128b .. 129+128b
            for fi in range(2):
                g0 = fi * 8
                xf = xpool.tile([P, WP], f32, tag="xf")
                for band in range(4):
                    nc.sync.dma_start(
                        out=xf[32 * band : 32 * (band + 1)],
                        in_=xpad[g0 : g0 + 8, 126 + 128 * band : 130 + 128 * band, :],
                    )
                bofs = (N_IMG + fi) * 18
                df_tiles = []
                for t in range(9):
                    i, j = divmod(t, 3)
                    d = dpool.tile([P, W], bf16, tag="d")
                    src = xf[:, j : j + W]
                    if t in ACT_TAPS:
                        nc.scalar.activation(
                            out=d,
                            in_=src,
                            func=mybir.ActivationFunctionType.Abs,
                            bias=b_all[:, bofs + 9 + t : bofs + 10 + t],
                            scale=1.0,
                        )
                    else:
                        nc.vector._custom_dve(
                            absdiff,
                            out=d,
                            in0=src,
                            s0=b_all[:, bofs + t : bofs + t + 1],
                        )
                    df_tiles.append(d)

                pf = ppool.tile([P, W], mybir.dt.float32, tag="ps")
                for i in range(3):
                    t0 = 3 * i
                    for j in range(3):
                        nc.tensor.matmul(
                            pf,
                            lhsT=s_t[:, i, :],
                            rhs=df_tiles[t0 + j],
                            start=(t0 + j == 0),
                            stop=(t0 + j == 8),
                        )

                of = opool.tile([P, W], f32, tag="o")
                nc.scalar.activation(
                    out=of,
                    in_=pf,
                    func=mybir.ActivationFunctionType.Copy,
                    scale=-1.0,
                )
                for band in range(4):
                    for g in range(8):
                        lo = 32 * band + 4 * g
                        nc.sync.dma_start(
                            out=outp[g0 + g, 127 + 128 * band : 129 + 128 * band, :],
                            in_=of[lo + 1 : lo + 3],
                        )
    nc.finalize()
    return nc


def _get_program():
    if "nc" not in _PROGRAM_CACHE:
        _PROGRAM_CACHE["nc"] = _build_program()
    return _PROGRAM_CACHE["nc"]


def _host_consts(weight):
    """Shift matrices + per-partition bias tables (shared by all cores)."""
    w9 = np.asarray(weight, np.float32).reshape(C, 9)  # [c, t]

    S = np.zeros((3, P, P), np.float32)
    for i in range(3):
        for p in range(P):
            k = p + i - 1
            if 0 <= k < P:
                S[i, k, p] = 1.0
    S = S.astype(ml_dtypes.bfloat16)

    # one preloaded bias table [P, (N_IMG+2)*18]:
    # cols st*18+t = +w (DVE taps), st*18+9+t = -w (ACT taps)
    bias = np.zeros((P, (N_IMG + 2) * 18), np.float32)
    for st in range(N_IMG):  # main tiles: channel st % C, all partitions equal
        c = st % C
        bias[:, st * 18 : st * 18 + 9] = w9[c][None, :]
        bias[:, st * 18 + 9 : st * 18 + 18] = -w9[c][None, :]
    for fi in range(2):  # fixup tiles: partition q = 32*band + 4*g + r
        o = (N_IMG + fi) * 18
        for band in range(4):
            for g in range(8):
                c = (fi * 8 + g) % C
                lo = 32 * band + 4 * g
                bias[lo : lo + 4, o : o + 9] = w9[c][None, :]
                bias[lo : lo + 4, o + 9 : o + 18] = -w9[c][None, :]
    return S, bias


def kernel(input, weight):
    from concourse.bass_utils import run_bass_kernel_spmd

    x = np.asarray(input, np.float32)
    S, bias = _host_consts(weight)

    xpad = np.pad(x, ((0, 0), (0, 0), (1, 1), (1, 1)))  # [B, C, HP, WP]
    in_maps = []
    for core in range(N_CORES):
        shard = np.ascontiguousarray(
            xpad[core * B_LOC : (core + 1) * B_LOC].reshape(N_IMG, HP, WP)
        )
        in_maps.append({"xpad": shard, "smat": S, "bias": bias})

    nc = _get_program()
    res = run_bass_kernel_spmd(nc, in_maps, core_ids=list(range(N_CORES)))

    out = np.empty((B, C, H, W), np.float32)
    for core in range(N_CORES):
        o = res.results[core]["outp"].reshape(B_LOC, C, HP, W)
        out[core * B_LOC : (core + 1) * B_LOC] = o[:, :, 1 : H + 1, :]
    return out
